# revision 1
# baseline (speedup 1.0000x reference)
"""AttentionNCF Trainium2 kernel (8-core SPMD, data-parallel over batch).

Math: reference computes
    scores[b,i] = cand[b]@w_c + rated[i]@w_r + b_att
    attn = softmax(where(user==0, -inf, scores), axis=i)
    user_est = (attn*user) @ rated ; then item/user towers + MLP.
Scores are rank-1 separable (a_b + r_i), so the per-row term a_b and b_att
cancel in the row softmax.  With v_i = exp(r_i):
    (attn*user)[b,i] = v_i * user[b,i] / s_b,   s_b = sum_i v_i * [user[b,i]!=0]
so attention is: wt = user * v (elementwise, v broadcast over b),
user_est[b,:] = (wt @ rated)[b,:] / s_b.  No (B,I) softmax passes needed.
All hidden-layer biases are jnp.zeros by construction -> omitted.

Design (v2, ~63-70us HW vs 73-79us for v1):
- Precision: attention data path bf16 (rated/userT/wt/v) with fp32 PSUM +
  fp32 softmax denominator; towers/MLP in fp16 (weights + activations) with
  fp32 PSUM accumulation.  fp16 (10-bit mantissa) keeps overall max-rel
  error ~1.7e-3 while halving weight DMA vs v1's fp32r (HBM read per core:
  17.4MB -> 11.3MB; DMA streams at the ~420GB/s bus wall either way).
- Towers are WEIGHT-STATIONARY: each layer computes hT = W.T @ xT directly
  (lhsT = 128x128 weight chunk, rhs = xT chunk), so every layer's output is
  produced already transposed for the next layer.  No PE transposes, no
  concat copy (m_w1 consumes ie/ue chunks in place), warm-PE cadence
  ~56ns/matmul with LDWEIGHTS hidden by the PE's pull-ahead queue.  Only
  user_est is transposed once (4 PE transposes) because the 1/s scaling
  must happen in batch-major orientation (per-partition scale).
- Score reductions r_c = sum_d rated_c*wr run on the DVE (STT at 1x,
  ~650ns/chunk = the attention-phase wall; measured alternatives all lose:
  gpsimd TT+DVE-reduce pays ~1.4us/op Pool overhead, the TS accum lowers
  to a 1x CACHE_REDUCE, and a cross-core AllGather of score columns costs
  ~50us in axon launch skew).  exp batches of 4 chunks pipeline DVE -> ACT
  (exp, wt scale) -> PE (est/s accumulation); the item tower is emitted
  mid-attention to keep the PE dense and HAM-warm.
- DMA: inputs are packed host-side into few large buffers so each dma_start
  (~650ns of HWDGE sequencing each) moves large >=1KB/partition descriptor
  sets: 9 paired (rated||userT) groups with graduated sizes + 3 weight
  packs ordered by first use on the sync queue; the cand/item-tower pack
  rides the scalar-engine HWDGE queue.
"""

from contextlib import ExitStack

import ml_dtypes
import numpy as np

import concourse.bass as bass
import concourse.mybir as mybir
import concourse.tile as tile
from concourse import bacc
from concourse.bass_utils import run_bass_kernel_spmd
from concourse.masks import make_identity

B, I, D = 1024, 4096, 512
IE, UE = 256, 512
D1, D2, D3, D4 = 1024, 512, 256, 128
NCORES = 8
BS = B // NCORES   # 128 batch rows per core
NI = I // 128      # 32 i-chunks
PAIR_SIZES = [1, 1, 2, 4, 4, 4, 4, 6, 6]  # rated+userT chunks per paired DMA
EXPB = 4                             # chunks per exp batch

f32 = mybir.dt.float32
f16 = mybir.dt.float16
bf16 = mybir.dt.bfloat16
AF = mybir.ActivationFunctionType
OP = mybir.AluOpType

# Tower layers: name -> (K, F); packed into 3 DMA buffers by first use.
LAYERS = {
    "ie_w1": (D, 2 * IE), "ie_w2": (2 * IE, IE),
    "ue_w1": (D, 2 * UE), "ue_w2": (2 * UE, UE),
    "m_w1": (IE + UE, D1), "m_w2": (D1, D2), "m_w3": (D2, D3),
    "m_w4": (D3, D4),
}
# pack name -> ordered layer list ("candT" is the transposed candidate input)
PACKS = {
    "cie": ["candT", "ie_w1", "ie_w2"],
    "wp1": ["ue_w1", "ue_w2"],
    "wp2": ["m_w1", "m_w2"],
    "wp3": ["m_w3", "m_w4", "w5"],
}
PACK_SHAPES = dict(LAYERS, candT=(D, BS), w5=(D4, 1))


def _pack_offsets():
    offs = {}
    for pack, names in PACKS.items():
        off = 0
        for n in names:
            K, F = PACK_SHAPES[n]
            offs[n] = (pack, off, K, F)
            off += (K // 128) * F
        offs[pack + "__total"] = off
    return offs


POFF = _pack_offsets()


def build_nc():
    nc = bacc.Bacc(
        "TRN2", target_bir_lowering=False, debug=False, num_devices=NCORES
    )

    wr = nc.dram_tensor("wr", [128, D], bf16, kind="ExternalInput").ap()
    pair_ap = []
    for g, n in enumerate(PAIR_SIZES):
        pair_ap.append(
            nc.dram_tensor(f"pair{g}", [128, n, D + BS], bf16,
                           kind="ExternalInput").ap())
    pk_ap = {}
    for pack in PACKS:
        pk_ap[pack] = nc.dram_tensor(
            pack, [128, POFF[pack + "__total"]], f16, kind="ExternalInput"
        ).ap()
    out = nc.dram_tensor("out", [BS, 1], f32, kind="ExternalOutput").ap()

    with tile.TileContext(nc) as tc, ExitStack() as ctx:
        pool = ctx.enter_context(tc.tile_pool(name="main", bufs=1))
        prod_v = ctx.enter_context(tc.tile_pool(name="prodv", bufs=2))
        wt_pool = ctx.enter_context(tc.tile_pool(name="wt", bufs=3))
        psum_att = ctx.enter_context(tc.tile_pool(name="psA", bufs=1, space="PSUM"))
        psum_s = ctx.enter_context(tc.tile_pool(name="psS", bufs=1, space="PSUM"))
        psum_layer = ctx.enter_context(tc.tile_pool(name="psL", bufs=2, space="PSUM"))
        psum_misc = ctx.enter_context(tc.tile_pool(name="psM", bufs=1, space="PSUM"))

        identity = pool.tile([128, 128], f16)
        make_identity(nc, identity[:])

        # ---- DMAs.  sync HWDGE queue: wr, paired rated||userT groups
        # (graduated sizes), then weight packs by first use.  scalar HWDGE
        # queue: the cand/item-tower pack.
        pk_tiles = {}

        def dma_pack(pack, engine):
            t = pool.tile([128, POFF[pack + "__total"]], f16, tag=pack)
            engine.dma_start(t[:], pk_ap[pack][:, :])
            pk_tiles[pack] = t

        wr_bc = pool.tile([128, D], bf16)
        nc.sync.dma_start(wr_bc[:], wr[:, :])
        dma_pack("cie", nc.scalar)

        rated_cs = [None] * NI   # per-chunk (128, D) bf16 APs
        ut_grp = []              # per-group (tile, c0, n)
        c0 = 0
        for g, n in enumerate(PAIR_SIZES):
            t = pool.tile([128, n, D + BS], bf16, tag=f"pair{g}")
            nc.sync.dma_start(t[:], pair_ap[g][:, :, :])
            for j in range(n):
                rated_cs[c0 + j] = t[:, j, :D]
            ut_grp.append((t, c0, n))
            c0 += n
        for pk in ("wp1", "wp2", "wp3"):
            dma_pack(pk, nc.sync)

        def wslice(name, k, f0, fn=128):
            pack, off, K, F = POFF[name]
            base = off + k * F + f0
            return pk_tiles[pack][:, base:base + fn]

        def ut_view(c):
            """(3D userT view (128, n, BS), group start, group len) for the
            pair group containing chunk c."""
            for t, g0, n in ut_grp:
                if g0 <= c < g0 + n:
                    return t[:, :, D:], g0, n
            raise AssertionError

        # ---- Scores (local, DVE-throughput-bound): r_c = sum_d rated_c*wr.
        v_all = pool.tile([128, NI], f32)
        v_bf = pool.tile([128, NI], bf16)

        # ---- Weight-stationary tower layer helper ----
        def wlayer(xT_chunks, wname, last_relu=True):
            """hT = relu(W.T @ x) with x given as K-major 128-chunks.
            Relu+copy PSUM->SBUF runs on the DVE (idle in the tower phase).
            Returns list of (128, BS) chunk APs of the output."""
            K, F = PACK_SHAPES[wname]
            nk = K // 128
            assert len(xT_chunks) == nk
            hT = pool.tile([128, F], f16, tag=f"h_{wname}")
            for f0 in range(0, F, 512):
                fn = min(512, F - f0)
                ps = psum_layer.tile([BS, fn], f32, tag="psL")
                for fs in range(0, fn, 128):
                    for k in range(nk):
                        nc.tensor.matmul(
                            ps[:, fs:fs + 128],
                            lhsT=wslice(wname, k, f0 + fs),
                            rhs=xT_chunks[k],
                            start=(k == 0), stop=(k == nk - 1),
                        )
                dst = hT[:, f0:f0 + fn]
                if last_relu:
                    nc.vector.tensor_scalar_max(dst, ps[:], 0.0)
                else:
                    nc.vector.tensor_copy(dst, ps[:])
            return [hT[:, j * 128:(j + 1) * 128] for j in range(F // 128)]

        candT_chunks = [wslice("candT", 0, j * 128) for j in range(D // 128)]
        item_out = {}

        def emit_h1():
            item_out["h1"] = wlayer(candT_chunks, "ie_w1")

        def emit_ie():
            item_out["ie"] = wlayer(item_out["h1"], "ie_w2")

        # ---- Attention: score STTs pipelined with exp batches, wt scales,
        # and est/s matmuls; item tower interleaved to keep the PE dense.
        est_psum = psum_att.tile([BS, D], f32)
        s_psum = psum_s.tile([BS, 1], f32)
        ind_tiles = {}   # group c0 -> ind tile (128, n, BS) bf16
        item_emits = {12: emit_h1, 20: emit_ie}

        for b0 in range(0, NI, EXPB):
            # per-batch score tile: a fresh tile per exp batch avoids a
            # false WAR between the next batch's accumulator writes and
            # the ACT exp read of the previous batch's columns.
            rcol_b = prod_v.tile([128, EXPB], f32, tag="rcol")
            for c in range(b0, b0 + EXPB):
                ut, g0, n = ut_view(c)
                if c == g0:
                    ind = wt_pool.tile([128, n, BS], bf16, tag=f"ind{g0}")
                    nc.vector.tensor_scalar(
                        ind[:, :, :], ut[:, :, :], 0.0, None, OP.is_gt
                    )
                    ind_tiles[g0] = ind
                acc = rcol_b[:, c - b0:c - b0 + 1]
                if c % 5 == 2:
                    # offload 6 reductions to the scalar engine's
                    # accumulator; DVE only multiplies (2x mode).  The TT
                    # output gets its own pool tag so later DVE multiplies
                    # never wait on ACT's reads (WAR on slot reuse).
                    prod = prod_v.tile([128, D], bf16, tag="pv")
                    nc.vector.tensor_tensor(
                        prod[:], rated_cs[c], wr_bc[:], OP.mult
                    )
                    junk = prod_v.tile([128, D], bf16, tag="pj")
                    nc.scalar.activation(
                        junk[:], prod[:], AF.Copy, accum_out=acc
                    )
                else:
                    prod = prod_v.tile([128, D], bf16, tag="pv")
                    nc.vector.scalar_tensor_tensor(
                        out=prod[:], in0=rated_cs[c], scalar=1.0,
                        in1=wr_bc[:],
                        op0=OP.mult, op1=OP.mult, accum_out=acc,
                    )
            sl = slice(b0, b0 + EXPB)
            nc.scalar.activation(v_all[:, sl], rcol_b[:], AF.Exp)
            nc.scalar.copy(v_bf[:, sl], v_all[:, sl])
            for c in range(b0, b0 + EXPB):
                ut, g0, n = ut_view(c)
                j = c - g0
                wt = wt_pool.tile([128, BS], bf16, tag="wt")
                nc.scalar.activation(
                    wt[:], ut[:, j, :], AF.Copy, scale=v_all[:, c:c + 1]
                )
                nc.tensor.matmul(
                    est_psum[:], lhsT=wt[:], rhs=rated_cs[c],
                    start=(c == 0), stop=(c == NI - 1),
                )
                nc.tensor.matmul(
                    s_psum[:], lhsT=ind_tiles[g0][:, j, :],
                    rhs=v_bf[:, c:c + 1],
                    start=(c == 0), stop=(c == NI - 1),
                )
            if b0 in item_emits:
                item_emits[b0]()

        s_eps = pool.tile([BS, 1], f32)
        nc.vector.tensor_scalar_add(s_eps[:], s_psum[:], 1e-30)
        recip = pool.tile([BS, 1], f32)
        nc.vector.reciprocal(recip[:], s_eps[:])
        est = pool.tile([BS, D], f16)
        nc.vector.tensor_scalar(est[:], est_psum[:], recip[:], None, OP.mult)

        # estT: one transpose set (scaling had to happen batch-major).
        tp = psum_layer.tile([128, D], f16, tag="psL")
        for j in range(4):
            nc.tensor.transpose(
                tp[:, j * 128:(j + 1) * 128],
                est[:, j * 128:(j + 1) * 128], identity[:],
            )
        estT = pool.tile([128, D], f16)
        nc.vector.tensor_copy(estT[:], tp[:])

        estT_chunks = [estT[:, j * 128:(j + 1) * 128] for j in range(4)]
        u1 = wlayer(estT_chunks, "ue_w1")
        u2 = wlayer(u1, "ue_w2")
        m1 = wlayer(item_out["ie"] + u2, "m_w1")
        m2 = wlayer(m1, "m_w2")
        m3 = wlayer(m2, "m_w3")
        m4 = wlayer(m3, "m_w4")
        out_ps = psum_misc.tile([BS, 1], f32, tag="misc")
        nc.tensor.matmul(
            out_ps[:], lhsT=m4[0], rhs=wslice("w5", 0, 0, fn=1),
            start=True, stop=True,
        )
        out_sb = pool.tile([BS, 1], f32)
        nc.vector.tensor_copy(out_sb[:], out_ps[:])
        nc.sync.dma_start(out[:, :], out_sb[:])

    nc.compile()
    return nc


_NC_CACHE = None


def get_nc():
    global _NC_CACHE
    if _NC_CACHE is None:
        _NC_CACHE = build_nc()
    return _NC_CACHE


def _shuffle(x, dtype):
    """(K, F) row-major -> (128, K/128, F) partition-major contiguous."""
    K, F = x.shape
    return np.ascontiguousarray(
        x.reshape(K // 128, 128, F).transpose(1, 0, 2).astype(dtype))


def make_in_maps(inputs):
    cand = np.asarray(inputs["candidate_items"], np.float32)
    rated = np.asarray(inputs["rated_items"], np.float32)
    user = np.asarray(inputs["user_matrix"], np.float32)
    w_att = np.asarray(inputs["w_att"], np.float32)
    wr = np.ascontiguousarray(np.broadcast_to(
        w_att[D:, 0].reshape(1, D).astype(ml_dtypes.bfloat16), (128, D)))
    rated_sh = _shuffle(rated, ml_dtypes.bfloat16)    # (128, NI, D)

    def pack(pname, mats):
        parts = []
        for name in PACKS[pname]:
            parts.append(_shuffle(mats[name], np.float16).reshape(128, -1))
        return np.ascontiguousarray(np.concatenate(parts, axis=1))

    shared_mats = {name: np.asarray(inputs[name], np.float32)
                   for name in LAYERS}
    shared_mats["w5"] = np.asarray(inputs["m_w5"], np.float32)
    shared = {
        "wr": wr,
        "wp1": pack("wp1", shared_mats),
        "wp2": pack("wp2", shared_mats),
        "wp3": pack("wp3", shared_mats),
    }

    in_maps = []
    for core in range(NCORES):
        sl = slice(core * BS, (core + 1) * BS)
        ut_sh = _shuffle(np.ascontiguousarray(user[sl].T),
                         ml_dtypes.bfloat16)              # (128, NI, BS)
        pairs = {}
        c0 = 0
        for g, n in enumerate(PAIR_SIZES):
            pairs[f"pair{g}"] = np.ascontiguousarray(np.concatenate([
                rated_sh[:, c0:c0 + n], ut_sh[:, c0:c0 + n]], axis=2))
            c0 += n
        mats = dict(shared_mats)
        mats["candT"] = np.ascontiguousarray(cand[sl].T)
        in_maps.append({
            "cie": pack("cie", mats),
            **pairs, **shared,
        })
    return in_maps


def kernel(**inputs) -> np.ndarray:
    nc = get_nc()
    res = run_bass_kernel_spmd(nc, make_in_maps(inputs), list(range(NCORES)))
    return np.concatenate([r["out"] for r in res.results], axis=0)

